# revision 3
# baseline (speedup 1.0000x reference)
"""Two-layer LSTM generator (B=4096, H=300, T=64) on 8 TRN2 NeuronCores, v2.

Same feature-on-partitions / batch-on-free mapping as v1, but the gate
M-columns are laid out per-layer so every TensorTensor pair shares a base
partition and every activation group is function-clustered:

- State s: ONE [128, 2560] bf16 tile; column-block k = K-chunk k (5 chunks
  of 128 K-rows, N=512 batch).  K rows: chunks 0-1 = slot0 (x/h1) rows
  0-255, chunk2 = [h1 256-299 | bias@44 | 0 | h0 0-63], chunk3 = h0 64-191,
  chunk4[0:108] = h0 192-299.
- Per-layer gate strips sized to the slot the layer's h writes into:
  layer0 (h0 -> slot1) strips (64,128,108); layer1 (h1 -> slot0) strips
  (128,128,44).  10 m-tiles/layer; tails share tiles pairwise (g|f, i|o)
  at bases (0,64) so i*g, f*c, o*th all pair legally.
- 5 PSUM groups of 2 banks (tag bufs=4): G0 = [T(i0|f0), T(g0|o0)] (the
  "early" piece's four gate strips), G1 = g mid/tail (tanh), G2 = i, G3 = f,
  G4 = o.  Fine-grained bank recycling keeps PE from ever waiting on ACT
  drains; the early piece's chain + h-write runs while G1-G4 stream.
- State is ping-pong buffered (cur/nxt per step): h-writes go to the other
  buffer, so no state WAR hazard ever blocks a matmul.  Layer 1 writes its
  h1 tail twice (nxt.2a deferred past its own k2 matmuls, plus cur.2a for
  the next step's layer-1 k2 read).
"""

import sys

if "/opt/trn_rl_repo" not in sys.path:
    sys.path.insert(0, "/opt/trn_rl_repo")

from contextlib import ExitStack

import numpy as np
import ml_dtypes

import concourse.bacc as bacc
import concourse.mybir as mybir
import concourse.tile as tile

F32 = mybir.dt.float32
BF16 = mybir.dt.bfloat16

H = 300
B = 512            # per-core batch
KK = 640           # 5 K-chunks of 128
NK = 5
N_CORES = 8
SIG = mybir.ActivationFunctionType.Sigmoid
TANH = mybir.ActivationFunctionType.Tanh

# ---------------------------------------------------------------------------
# Per-layer layout tables.
# strip s of gate G lives at (tau, p0, p1) in the 10-tile M space.
# Layer 0: strips (0-64, 64-192, 192-300);  Layer 1: (0-128, 128-256, 256-300)
L0 = dict(
    strips=[(0, 64), (64, 192), (192, 300)],
    g=[(1, 0, 64), (2, 0, 128), (3, 0, 108)],
    i=[(0, 0, 64), (4, 0, 128), (5, 0, 108)],
    f=[(0, 64, 128), (6, 0, 128), (7, 0, 108)],
    o=[(1, 64, 128), (8, 0, 128), (9, 0, 108)],
    cell=[(64, 128), (0, 128), (0, 108)],
    hdst=[(2, 64, 128), (3, 0, 128), (4, 0, 108)],
    acts=[[(SIG, 0, 128, 0, 512), (TANH, 0, 64, 512, 1024),
           (SIG, 64, 128, 512, 1024)],
          [(TANH, 0, 128, 1024, 2048)],
          [(SIG, 0, 128, 2048, 3072)],
          [(SIG, 0, 128, 3072, 4096)],
          [(SIG, 0, 128, 4096, 4608), (SIG, 0, 128, 4608, 5120)]],
    korder=(3, 4, 2, 0, 1),
    early=0,
)
L1 = dict(
    strips=[(0, 128), (128, 256), (256, 300)],
    g=[(2, 0, 128), (3, 0, 128), (1, 0, 44)],
    i=[(4, 0, 128), (5, 0, 128), (0, 0, 44)],
    f=[(6, 0, 128), (7, 0, 128), (0, 64, 108)],
    o=[(8, 0, 128), (9, 0, 128), (1, 64, 108)],
    cell=[(0, 128), (0, 128), (64, 108)],
    hdst=[(0, 0, 128), (1, 0, 128), (2, 0, 44)],
    acts=[[(SIG, 0, 128, 0, 512), (TANH, 0, 44, 512, 1024),
           (SIG, 64, 108, 512, 1024)],
          [(TANH, 0, 128, 1024, 2048)],
          [(SIG, 0, 128, 2048, 3072)],
          [(SIG, 0, 128, 3072, 4096)],
          [(SIG, 0, 128, 4096, 4608), (SIG, 0, 128, 4608, 5120)]],
    korder=(0, 1, 2, 3, 4),
    early=2,
)
LAYOUTS = (L0, L1)
GROUPS = [(0, 1), (2, 3), (4, 5), (6, 7), (8, 9)]


def _emit_body(nc, s_pp, c_sb, w_sb, w1f_sb, gates_pool, cell_pool, psum_pool,
               yT, T, gate_dt, timed):
    def scol(k):
        return slice(512 * k, 512 * (k + 1))

    for t in range(T):
        cur, nxt = s_pp[t % 2], s_pp[(t + 1) % 2]

        for l in range(2):
            LY = LAYOUTS[l]
            w = w_sb[l]
            if l == 1 and t == 0 and not timed:
                kws = [(2, w1f_sb), (3, w[3]), (4, w[4])]
            else:
                kws = [(k, w[k]) for k in LY["korder"]]

            # layer0 reads everything from cur; layer1 reads the fresh h0
            # chunks (2,3,4) from nxt, the old-h1 chunks (0,1) from cur
            def src(k):
                sb = nxt if (l == 1 and k >= 2) else cur
                return sb[:, scol(k)]

            def mm_group(pt, taus, kws):
                for ki, (k, wk) in enumerate(kws):
                    for j, tau in enumerate(taus):
                        nc.tensor.matmul(
                            pt[:, 512 * j:512 * (j + 1)],
                            wk[:, 128 * tau:128 * (tau + 1)],
                            src(k),
                            start=(ki == 0),
                            stop=(ki == len(kws) - 1),
                        )

            g_sb = gates_pool.tile([128, 5120], gate_dt, name="g", tag="g")

            def gview(st):
                tau, p0, p1 = st
                return g_sb[p0:p1, 512 * tau:512 * (tau + 1)]

            def emit_group(gi):
                pt = psum_pool.tile([128, 1024], F32, name=f"ps{gi}",
                                    tag="ps")
                mm_group(pt, GROUPS[gi], kws)
                base = 1024 * gi
                for func, p0, p1, c0, c1 in LY["acts"][gi]:
                    nc.scalar.activation(g_sb[p0:p1, c0:c1],
                                         pt[p0:p1, c0 - base:c1 - base],
                                         func)

            c = c_sb[l]
            t1 = cell_pool.tile([128, 1536], gate_dt, name="t1", tag="t1")
            t2 = cell_pool.tile([128, 1536], gate_dt, name="t2", tag="t2")
            th = cell_pool.tile([128, 1536], BF16, name="th", tag="th")

            def cpiece(x, b):
                p0, p1 = LY["cell"][b]
                return x[p0:p1, 512 * b:512 * (b + 1)]

            def chain(b):
                # t1 = i*g ; t2 = f*c ; c = t2+t1 ; th = tanh(c)
                nc.vector.tensor_mul(cpiece(t1, b), gview(LY["i"][b]),
                                     gview(LY["g"][b]))
                nc.vector.tensor_mul(cpiece(t2, b), gview(LY["f"][b]),
                                     cpiece(c, b))
                nc.vector.tensor_add(cpiece(c, b), cpiece(t2, b),
                                     cpiece(t1, b))
                nc.scalar.activation(cpiece(th, b), cpiece(c, b), TANH)

            def hmul(b, dst=None):
                k, p0, p1 = LY["hdst"][b]
                dst = nxt if dst is None else dst
                nc.vector.tensor_mul(dst[p0:p1, scol(k)],
                                     gview(LY["o"][b]), cpiece(th, b))

            # G0 holds every gate strip of the "early" piece: its whole
            # chain + h-write runs while G1-G4 matmuls stream.  With
            # ping-pong state the early h-write has no WAR hazard.
            early = LY["early"]
            emit_group(0)
            chain(early)
            if l == 0:
                hmul(0)
            else:
                # h1 tail for next step's l1 (reads it from its cur = our cur)
                hmul(2, dst=cur)
            emit_group(1)
            emit_group(2)
            emit_group(3)
            late = [b for b in range(3) if b != early]
            for b in late:
                chain(b)
            emit_group(4)
            if l == 1:
                # deferred: nxt.2a read (old h1 tail) by this layer's own k2
                # matmuls is complete only after G4's k-loop
                hmul(2)
            for b in late:
                hmul(b)

        td = t % yT.shape[0]
        nc.sync.dma_start(out=yT[td, 0:128, :], in_=nxt[:, 0:512])
        nc.sync.dma_start(out=yT[td, 128:256, :], in_=nxt[:, 512:1024])
        nc.sync.dma_start(out=yT[td, 256:300, :], in_=nxt[0:44, 1024:1536])


def _build(T, gate_dt, out_dt, timed_reps=None):
    nc = bacc.Bacc(None, target_bir_lowering=False)
    TB = T if timed_reps is None else T

    w0 = nc.dram_tensor("w0", [KK, 1280], BF16, kind="ExternalInput")
    w1 = nc.dram_tensor("w1", [KK, 1280], BF16, kind="ExternalInput")
    w1f = nc.dram_tensor("w1f", [128, 1280], BF16, kind="ExternalInput")
    xz = nc.dram_tensor("xz", [320, B], BF16, kind="ExternalInput")
    yT = nc.dram_tensor("yT", [TB, H, B], out_dt, kind="ExternalOutput")

    with tile.TileContext(nc) as tc, ExitStack() as ctx:
        persist = ctx.enter_context(tc.tile_pool(name="persist", bufs=1))
        gates_pool = ctx.enter_context(tc.tile_pool(name="gates", bufs=3))
        cell_pool = ctx.enter_context(tc.tile_pool(name="cell", bufs=3))
        psum_pool = ctx.enter_context(
            tc.tile_pool(name="psum", bufs=4, space="PSUM"))

        w_sb = [[persist.tile([128, 1280], BF16, name=f"w{l}_{k}",
                              tag=f"w{l}_{k}") for k in range(NK)]
                for l in range(2)]
        w1f_sb = persist.tile([128, 1280], BF16, name="w1f", tag="w1f")
        s_pp = [persist.tile([128, 2560], BF16, name=f"s{p}", tag=f"s{p}")
                for p in range(2)]
        c_sb = [persist.tile([128, 1536], F32, name=f"c{l}", tag=f"c{l}")
                for l in range(2)]

        for l, w, ks in ((0, w0, L0["korder"]), (1, w1, L1["korder"])):
            for k in ks:
                nc.sync.dma_start(out=w_sb[l][k],
                                  in_=w[128 * k:128 * (k + 1), :])
        nc.sync.dma_start(out=w1f_sb, in_=w1f[:, :])
        # both buffers: chunks 0,1 + chunk2[0:64] from xz; rest zero
        for sb in s_pp:
            nc.sync.dma_start(out=sb[:, 0:512], in_=xz[0:128, :])
            nc.sync.dma_start(out=sb[:, 512:1024], in_=xz[128:256, :])
            nc.sync.dma_start(out=sb[0:64, 1024:1536], in_=xz[256:320, :])
            nc.vector.memset(sb[64:128, 1024:1536], 0.0)
            nc.vector.memset(sb[:, 1536:2560], 0.0)
        nc.vector.memset(c_sb[0], 0.0)
        nc.vector.memset(c_sb[1], 0.0)

        if timed_reps is None:
            _emit_body(nc, s_pp, c_sb, w_sb, w1f_sb, gates_pool, cell_pool,
                       psum_pool, yT, T, F32, timed=False)
        else:
            with tc.For_i(0, timed_reps):
                _emit_body(nc, s_pp, c_sb, w_sb, w1f_sb, gates_pool,
                           cell_pool, psum_pool, yT, T, F32, timed=True)
    return nc


def build_nc(T, gate_dt=F32, out_dt=BF16):
    return _build(T, gate_dt, out_dt)


def build_timed(TB, reps):
    return _build(TB, F32, BF16, timed_reps=reps)


# ---------------------------------------------------------------------------
# Host-side weight packing
_GOFF = {"i": 0, "f": 300, "g": 600, "o": 900}


def _krows():
    """K row -> (slot, feat) with slot -1 = invalid, -2 = bias."""
    out = []
    for c in range(5):
        for p in range(128):
            if c < 2:
                out.append((0, 128 * c + p))
            elif c == 2:
                if p < 44:
                    out.append((0, 256 + p))
                elif p == 44:
                    out.append((-2, 0))
                elif p < 64:
                    out.append((-1, 0))
                else:
                    out.append((1, p - 64))
            elif c == 3:
                out.append((1, 64 + p))
            else:
                out.append((1, 192 + p) if p < 108 else (-1, 0))
    return out


def _mcols(LY):
    """M col -> (torch_row, valid) via gate strip tables."""
    rows = np.full(1280, -1, np.int64)
    for gname in "gifo":
        for s, (tau, p0, p1) in enumerate(LY[gname]):
            r0, r1 = LY["strips"][s]
            assert (p1 - p0) == (r1 - r0)
            rows[128 * tau + p0:128 * tau + p1] = (
                _GOFF[gname] + np.arange(r0, r1))
    return rows


def _pack_w(l, Ws0, Ws1, b):
    """Ws0 multiplies slot0 (x/h1), Ws1 slot1 (h0); b = b_ih+b_hh."""
    kr = _krows()
    mrows = _mcols(LAYOUTS[l])
    out = np.zeros((KK, 1280), np.float32)
    mv = np.nonzero(mrows >= 0)[0]
    gr = mrows[mv]
    k0 = np.array([i for i, (s, f) in enumerate(kr) if s == 0])
    f0 = np.array([f for s, f in kr if s == 0])
    k1 = np.array([i for i, (s, f) in enumerate(kr) if s == 1])
    f1 = np.array([f for s, f in kr if s == 1])
    out[np.ix_(k0, mv)] = Ws0[np.ix_(gr, f0)].T
    out[np.ix_(k1, mv)] = Ws1[np.ix_(gr, f1)].T
    out[300, mv] = b[gr]
    return out.astype(ml_dtypes.bfloat16)


def _prep_shared(W_ih0, W_hh0, b0, W_ih1, W_hh1, b1):
    w0 = _pack_w(0, W_ih0, W_hh0, b0)
    w1 = _pack_w(1, W_hh1, W_ih1, b1)
    # layer1 t=0: kill slot0 (h1-init=0) contributions in chunk2; keep bias
    # row (p=44) and slot1 rows (p>=64)
    w1f = np.array(w1[256:384], np.float32)
    w1f[0:44] = 0.0
    w1f[45:64] = 0.0
    return w0, w1, w1f.astype(ml_dtypes.bfloat16)


def prep_core_inputs(z_shard, W_ih0, W_hh0, b0, W_ih1, W_hh1, b1):
    w0, w1, w1f = _prep_shared(W_ih0, W_hh0, b0, W_ih1, W_hh1, b1)
    xz = np.zeros((320, B), np.float32)
    xz[0:H, :] = z_shard.T
    xz[300, :] = 1.0
    return {"w0": w0, "w1": w1, "w1f": w1f,
            "xz": xz.astype(ml_dtypes.bfloat16)}


_NC_CACHE = {}
last_results = None


def kernel(z, W_ih0, W_hh0, b_ih0, b_hh0, W_ih1, W_hh1, b_ih1, b_hh1,
           sentence_len):
    global last_results
    from concourse.bass_utils import run_bass_kernel_spmd

    T = int(sentence_len)
    if T not in _NC_CACHE:
        nc = build_nc(T)
        nc.compile()
        _NC_CACHE[T] = nc
    nc = _NC_CACHE[T]

    z = np.asarray(z, np.float32)
    b0 = np.asarray(b_ih0, np.float32) + np.asarray(b_hh0, np.float32)
    b1 = np.asarray(b_ih1, np.float32) + np.asarray(b_hh1, np.float32)
    in_maps = [prep_core_inputs(z[i * B:(i + 1) * B],
                                np.asarray(W_ih0, np.float32),
                                np.asarray(W_hh0, np.float32), b0,
                                np.asarray(W_ih1, np.float32),
                                np.asarray(W_hh1, np.float32), b1)
               for i in range(N_CORES)]

    last_results = run_bass_kernel_spmd(
        nc, in_maps, core_ids=list(range(N_CORES)))

    out = np.empty((N_CORES * B, 1, T, H), np.float32)
    for i, r in enumerate(last_results.results):
        yT = np.asarray(r["yT"])  # [T, 300, 512] bf16
        u32 = yT.view(np.uint16).astype(np.uint32) << 16
        out[i * B:(i + 1) * B, 0] = (
            u32.view(np.float32).transpose(2, 0, 1))
    return out


# revision 4
# speedup vs baseline: 1.1046x; 1.1046x over previous
"""Two-layer LSTM generator (B=4096, H=300, T=64) on 8 TRN2 NeuronCores, v2.

Same feature-on-partitions / batch-on-free mapping as v1, but the gate
M-columns are laid out per-layer so every TensorTensor pair shares a base
partition and every activation group is function-clustered:

- State s: ONE [128, 2560] bf16 tile; column-block k = K-chunk k (5 chunks
  of 128 K-rows, N=512 batch).  K rows: chunks 0-1 = slot0 (x/h1) rows
  0-255, chunk2 = [h1 256-299 | bias@44 | 0 | h0 0-63], chunk3 = h0 64-191,
  chunk4[0:108] = h0 192-299.
- Per-layer gate strips sized to the slot the layer's h writes into:
  layer0 (h0 -> slot1) strips (64,128,108); layer1 (h1 -> slot0) strips
  (128,128,44).  10 m-tiles/layer; tails share tiles pairwise (g|f, i|o)
  at bases (0,64) so i*g, f*c, o*th all pair legally.
- 5 PSUM groups of 2 banks (tag bufs=4): G0 = [T(i0|f0), T(g0|o0)] (the
  "early" piece's four gate strips), G1 = g mid/tail (tanh), G2 = i, G3 = f,
  G4 = o.  Fine-grained bank recycling keeps PE from ever waiting on ACT
  drains; the early piece's chain + h-write runs while G1-G4 stream.
- State is ping-pong buffered (cur/nxt per step): h-writes go to the other
  buffer, so no state WAR hazard ever blocks a matmul.  Layer 1 writes its
  h1 tail twice (nxt.2a deferred past its own k2 matmuls, plus cur.2a for
  the next step's layer-1 k2 read).
"""

import sys

if "/opt/trn_rl_repo" not in sys.path:
    sys.path.insert(0, "/opt/trn_rl_repo")

from contextlib import ExitStack

import numpy as np
import ml_dtypes

import concourse.bacc as bacc
import concourse.mybir as mybir
import concourse.tile as tile

F32 = mybir.dt.float32
BF16 = mybir.dt.bfloat16

H = 300
B = 512            # per-core batch
KK = 640           # 5 K-chunks of 128
NK = 5
N_CORES = 8
SIG = mybir.ActivationFunctionType.Sigmoid
TANH = mybir.ActivationFunctionType.Tanh

# ---------------------------------------------------------------------------
# Per-layer layout tables.
# strip s of gate G lives at (tau, p0, p1) in the 10-tile M space.
# Layer 0: strips (0-64, 64-192, 192-300);  Layer 1: (0-128, 128-256, 256-300)
L0 = dict(
    strips=[(0, 64), (64, 192), (192, 300)],
    g=[(1, 0, 64), (2, 0, 128), (3, 0, 108)],
    i=[(0, 0, 64), (4, 0, 128), (5, 0, 108)],
    f=[(0, 64, 128), (6, 0, 128), (7, 0, 108)],
    o=[(1, 64, 128), (8, 0, 128), (9, 0, 108)],
    cell=[(64, 128), (0, 128), (0, 108)],
    hdst=[(2, 64, 128), (3, 0, 128), (4, 0, 108)],
    acts=[[(SIG, 0, 128, 0, 512), (TANH, 0, 64, 512, 1024),
           (SIG, 64, 128, 512, 1024)],
          [(TANH, 0, 128, 1024, 2048)],
          [(SIG, 0, 128, 2048, 3072)],
          [(SIG, 0, 128, 3072, 4096)],
          [(SIG, 0, 128, 4096, 4608), (SIG, 0, 128, 4608, 5120)]],
    korder=(3, 4, 2, 0, 1),
    early=0,
)
L1 = dict(
    strips=[(0, 128), (128, 256), (256, 300)],
    g=[(2, 0, 128), (3, 0, 128), (1, 0, 44)],
    i=[(4, 0, 128), (5, 0, 128), (0, 0, 44)],
    f=[(6, 0, 128), (7, 0, 128), (0, 64, 108)],
    o=[(8, 0, 128), (9, 0, 128), (1, 64, 108)],
    cell=[(0, 128), (0, 128), (64, 108)],
    hdst=[(0, 0, 128), (1, 0, 128), (2, 0, 44)],
    acts=[[(SIG, 0, 128, 0, 512), (TANH, 0, 44, 512, 1024),
           (SIG, 64, 108, 512, 1024)],
          [(TANH, 0, 128, 1024, 2048)],
          [(SIG, 0, 128, 2048, 3072)],
          [(SIG, 0, 128, 3072, 4096)],
          [(SIG, 0, 128, 4096, 4608), (SIG, 0, 128, 4608, 5120)]],
    korder=(0, 1, 2, 3, 4),
    early=2,
)
LAYOUTS = (L0, L1)
GROUPS = [(0, 1), (2, 3), (4, 5), (6, 7), (8, 9)]


def _emit_body(nc, s_pp, c_sb, w_sb, w1f_sb, gates_pool, cell_pool, psum_pool,
               yT, T, gate_dt, timed):
    def scol(k):
        return slice(512 * k, 512 * (k + 1))

    for t in range(T):
        cur, nxt = s_pp[t % 2], s_pp[(t + 1) % 2]

        for l in range(2):
            LY = LAYOUTS[l]
            w = w_sb[l]
            if l == 1 and t == 0 and not timed:
                kws = [(2, w1f_sb), (3, w[3]), (4, w[4])]
            else:
                kws = [(k, w[k]) for k in LY["korder"]]

            # layer0 reads everything from cur; layer1 reads the fresh h0
            # chunks (2,3,4) from nxt, the old-h1 chunks (0,1) from cur
            def src(k):
                sb = nxt if (l == 1 and k >= 2) else cur
                return sb[:, scol(k)]

            def mm_group(pt, taus, kws):
                for ki, (k, wk) in enumerate(kws):
                    for j, tau in enumerate(taus):
                        nc.tensor.matmul(
                            pt[:, 512 * j:512 * (j + 1)],
                            wk[:, 128 * tau:128 * (tau + 1)],
                            src(k),
                            start=(ki == 0),
                            stop=(ki == len(kws) - 1),
                        )

            g_sb = gates_pool.tile([128, 5120], gate_dt, name="g", tag="g")

            def gview(st):
                tau, p0, p1 = st
                return g_sb[p0:p1, 512 * tau:512 * (tau + 1)]

            def emit_group(gi):
                pt = psum_pool.tile([128, 1024], F32, name=f"ps{gi}",
                                    tag="ps")
                mm_group(pt, GROUPS[gi], kws)
                base = 1024 * gi
                for func, p0, p1, c0, c1 in LY["acts"][gi]:
                    nc.scalar.activation(g_sb[p0:p1, c0:c1],
                                         pt[p0:p1, c0 - base:c1 - base],
                                         func)

            c = c_sb[l]
            t1 = cell_pool.tile([128, 1536], gate_dt, name="t1", tag="t1")
            t2 = cell_pool.tile([128, 1536], gate_dt, name="t2", tag="t2")
            th = cell_pool.tile([128, 1536], BF16, name="th", tag="th")

            def cpiece(x, b):
                p0, p1 = LY["cell"][b]
                return x[p0:p1, 512 * b:512 * (b + 1)]

            def chain(b):
                # t1 = i*g ; t2 = f*c ; c = t2+t1 ; th = tanh(c)
                nc.vector.tensor_mul(cpiece(t1, b), gview(LY["i"][b]),
                                     gview(LY["g"][b]))
                nc.vector.tensor_mul(cpiece(t2, b), gview(LY["f"][b]),
                                     cpiece(c, b))
                nc.vector.tensor_add(cpiece(c, b), cpiece(t2, b),
                                     cpiece(t1, b))
                nc.scalar.activation(cpiece(th, b), cpiece(c, b), TANH)

            def hmul(b, dst=None):
                k, p0, p1 = LY["hdst"][b]
                dst = nxt if dst is None else dst
                nc.vector.tensor_mul(dst[p0:p1, scol(k)],
                                     gview(LY["o"][b]), cpiece(th, b))

            # G0 holds every gate strip of the "early" piece: its whole
            # chain + h-write runs while G1-G4 matmuls stream.  With
            # ping-pong state the early h-write has no WAR hazard.
            early = LY["early"]
            emit_group(0)
            chain(early)
            if l == 0:
                hmul(0)
            else:
                # h1 tail for next step's l1 (reads it from its cur = our cur)
                hmul(2, dst=cur)
            emit_group(1)
            emit_group(2)
            emit_group(3)
            late = [b for b in range(3) if b != early]
            for b in late:
                chain(b)
            emit_group(4)
            if l == 1:
                # deferred: nxt.2a read (old h1 tail) by this layer's own k2
                # matmuls is complete only after G4's k-loop
                hmul(2)
            for b in late:
                hmul(b)

        td = t % yT.shape[0]
        nc.sync.dma_start(out=yT[td, 0:128, :], in_=nxt[:, 0:512])
        nc.sync.dma_start(out=yT[td, 128:256, :], in_=nxt[:, 512:1024])
        nc.sync.dma_start(out=yT[td, 256:300, :], in_=nxt[0:44, 1024:1536])


def _build(T, gate_dt, out_dt, timed_reps=None):
    nc = bacc.Bacc(None, target_bir_lowering=False)
    TB = T if timed_reps is None else T

    w0 = nc.dram_tensor("w0", [KK, 1280], BF16, kind="ExternalInput")
    w1 = nc.dram_tensor("w1", [KK, 1280], BF16, kind="ExternalInput")
    w1f = nc.dram_tensor("w1f", [128, 1280], BF16, kind="ExternalInput")
    xz = nc.dram_tensor("xz", [320, B], BF16, kind="ExternalInput")
    yT = nc.dram_tensor("yT", [TB, H, B], out_dt, kind="ExternalOutput")

    with tile.TileContext(nc) as tc, ExitStack() as ctx:
        persist = ctx.enter_context(tc.tile_pool(name="persist", bufs=1))
        gates_pool = ctx.enter_context(tc.tile_pool(name="gates", bufs=3))
        cell_pool = ctx.enter_context(tc.tile_pool(name="cell", bufs=3))
        psum_pool = ctx.enter_context(
            tc.tile_pool(name="psum", bufs=4, space="PSUM"))

        w_sb = [[persist.tile([128, 1280], BF16, name=f"w{l}_{k}",
                              tag=f"w{l}_{k}") for k in range(NK)]
                for l in range(2)]
        w1f_sb = persist.tile([128, 1280], BF16, name="w1f", tag="w1f")
        s_pp = [persist.tile([128, 2560], BF16, name=f"s{p}", tag=f"s{p}")
                for p in range(2)]
        c_sb = [persist.tile([128, 1536], F32, name=f"c{l}", tag=f"c{l}")
                for l in range(2)]

        # small state-init DMAs first so step 0 isn't queued behind the
        # 3.3MB weight load; then weight chunks in first-use order
        for sb in s_pp:
            nc.sync.dma_start(out=sb[:, 0:512], in_=xz[0:128, :])
            nc.sync.dma_start(out=sb[:, 512:1024], in_=xz[128:256, :])
            nc.sync.dma_start(out=sb[0:64, 1024:1536], in_=xz[256:320, :])
            nc.vector.memset(sb[64:128, 1024:1536], 0.0)
            nc.vector.memset(sb[:, 1536:2560], 0.0)
        for l, w, ks in ((0, w0, L0["korder"]), (1, w1, L1["korder"])):
            for k in ks:
                nc.sync.dma_start(out=w_sb[l][k],
                                  in_=w[128 * k:128 * (k + 1), :])
        nc.sync.dma_start(out=w1f_sb, in_=w1f[:, :])
        nc.vector.memset(c_sb[0], 0.0)
        nc.vector.memset(c_sb[1], 0.0)

        if timed_reps is None:
            _emit_body(nc, s_pp, c_sb, w_sb, w1f_sb, gates_pool, cell_pool,
                       psum_pool, yT, T, F32, timed=False)
        else:
            with tc.For_i(0, timed_reps):
                _emit_body(nc, s_pp, c_sb, w_sb, w1f_sb, gates_pool,
                           cell_pool, psum_pool, yT, T, F32, timed=True)
    return nc


def build_nc(T, gate_dt=F32, out_dt=BF16):
    return _build(T, gate_dt, out_dt)


def build_timed(TB, reps):
    return _build(TB, F32, BF16, timed_reps=reps)


# ---------------------------------------------------------------------------
# Host-side weight packing
_GOFF = {"i": 0, "f": 300, "g": 600, "o": 900}


def _krows():
    """K row -> (slot, feat) with slot -1 = invalid, -2 = bias."""
    out = []
    for c in range(5):
        for p in range(128):
            if c < 2:
                out.append((0, 128 * c + p))
            elif c == 2:
                if p < 44:
                    out.append((0, 256 + p))
                elif p == 44:
                    out.append((-2, 0))
                elif p < 64:
                    out.append((-1, 0))
                else:
                    out.append((1, p - 64))
            elif c == 3:
                out.append((1, 64 + p))
            else:
                out.append((1, 192 + p) if p < 108 else (-1, 0))
    return out


def _mcols(LY):
    """M col -> (torch_row, valid) via gate strip tables."""
    rows = np.full(1280, -1, np.int64)
    for gname in "gifo":
        for s, (tau, p0, p1) in enumerate(LY[gname]):
            r0, r1 = LY["strips"][s]
            assert (p1 - p0) == (r1 - r0)
            rows[128 * tau + p0:128 * tau + p1] = (
                _GOFF[gname] + np.arange(r0, r1))
    return rows


def _pack_w(l, Ws0, Ws1, b):
    """Ws0 multiplies slot0 (x/h1), Ws1 slot1 (h0); b = b_ih+b_hh."""
    kr = _krows()
    mrows = _mcols(LAYOUTS[l])
    out = np.zeros((KK, 1280), np.float32)
    mv = np.nonzero(mrows >= 0)[0]
    gr = mrows[mv]
    k0 = np.array([i for i, (s, f) in enumerate(kr) if s == 0])
    f0 = np.array([f for s, f in kr if s == 0])
    k1 = np.array([i for i, (s, f) in enumerate(kr) if s == 1])
    f1 = np.array([f for s, f in kr if s == 1])
    out[np.ix_(k0, mv)] = Ws0[np.ix_(gr, f0)].T
    out[np.ix_(k1, mv)] = Ws1[np.ix_(gr, f1)].T
    out[300, mv] = b[gr]
    return out.astype(ml_dtypes.bfloat16)


def _prep_shared(W_ih0, W_hh0, b0, W_ih1, W_hh1, b1):
    w0 = _pack_w(0, W_ih0, W_hh0, b0)
    w1 = _pack_w(1, W_hh1, W_ih1, b1)
    # layer1 t=0: kill slot0 (h1-init=0) contributions in chunk2; keep bias
    # row (p=44) and slot1 rows (p>=64)
    w1f = np.array(w1[256:384], np.float32)
    w1f[0:44] = 0.0
    w1f[45:64] = 0.0
    return w0, w1, w1f.astype(ml_dtypes.bfloat16)


def prep_core_inputs(z_shard, W_ih0, W_hh0, b0, W_ih1, W_hh1, b1):
    w0, w1, w1f = _prep_shared(W_ih0, W_hh0, b0, W_ih1, W_hh1, b1)
    xz = np.zeros((320, B), np.float32)
    xz[0:H, :] = z_shard.T
    xz[300, :] = 1.0
    return {"w0": w0, "w1": w1, "w1f": w1f,
            "xz": xz.astype(ml_dtypes.bfloat16)}


_NC_CACHE = {}
last_results = None


def kernel(z, W_ih0, W_hh0, b_ih0, b_hh0, W_ih1, W_hh1, b_ih1, b_hh1,
           sentence_len):
    global last_results
    from concourse.bass_utils import run_bass_kernel_spmd

    T = int(sentence_len)
    if T not in _NC_CACHE:
        nc = build_nc(T)
        nc.compile()
        _NC_CACHE[T] = nc
    nc = _NC_CACHE[T]

    z = np.asarray(z, np.float32)
    b0 = np.asarray(b_ih0, np.float32) + np.asarray(b_hh0, np.float32)
    b1 = np.asarray(b_ih1, np.float32) + np.asarray(b_hh1, np.float32)
    in_maps = [prep_core_inputs(z[i * B:(i + 1) * B],
                                np.asarray(W_ih0, np.float32),
                                np.asarray(W_hh0, np.float32), b0,
                                np.asarray(W_ih1, np.float32),
                                np.asarray(W_hh1, np.float32), b1)
               for i in range(N_CORES)]

    last_results = run_bass_kernel_spmd(
        nc, in_maps, core_ids=list(range(N_CORES)))

    out = np.empty((N_CORES * B, 1, T, H), np.float32)
    for i, r in enumerate(last_results.results):
        yT = np.asarray(r["yT"])  # [T, 300, 512] bf16
        u32 = yT.view(np.uint16).astype(np.uint32) << 16
        out[i * B:(i + 1) * B, 0] = (
            u32.view(np.float32).transpose(2, 0, 1))
    return out
